# revision 33
# baseline (speedup 1.0000x reference)
"""DeepSpeed-style MLP block (pre-LN residual add + LN + GEMM+GELU + GEMM +
residual) for Trainium2, data-parallel over tokens across 8 NeuronCores.

Per-core pipeline (tokens sharded 8 x 4096, processed in 512-token blocks):
  r   = (input + bias) + residual                    (bias folded on host; DVE)
  x0  = (r - mean(r)) * rsqrt(var(r) + eps)          (bf16; LN affine folded
                                                      into W1/b1 on the host)
  xT  = PE-transpose(x0) -> fp8                      (grouped: 8 chunks share
                                                      one PSUM bank, one DVE
                                                      copy evacuates them)
  hT  = gelu_tanh(W1'-chunks.T @ xT / 64 + b1')      (PE fp8 DoubleRow + ACT)
  out = (hT-chunks.T @ W2) / 64 + r + output_b       (PE fp8 DoubleRow + DVE;
                                                      output_b added on gpsimd)

Both GEMMs run in fp8 e4m3 DoubleRow mode (256-row contraction per pass, 2x
bf16 throughput, ~215 ns per 256x128x512 matmul). Weights are scaled by 64 on
the host so their ~0.02-scale values sit in e4m3's normal range; the 1/64
rescale folds into the GELU activation scale (GEMM1) and the PSUM-evict
multiply (GEMM2). W1/W2 are SBUF-resident fp8 (32 KB/partition each, one
contiguous DMA each on the ACT HWDGE queue; W2's trigger is deferred past
block-0's LN so it doesn't steal startup bandwidth).

The per-block LN variances are batched into ONE ACT sqrt so the ACT function
table stays on GELU (interleaved sqrt/gelu reloads cost 1.3 us each). DVE
work is emitted in deadline order: next block's LN stats first, then x0
interleaved with the n=0 PSUM evictions, then the transpose copy-outs, then
the n=1 evictions, so the in-order DVE queue never head-of-line blocks a
PSUM-bank WAR the PE is waiting on. PE stream per block: GEMM1 | GEMM2[n=0] |
transposes(tb+1) | GEMM2[n=1]. Measured rel err ~1.8e-2 vs the fp32
reference (validated bit-close on the exact harness inputs host-side).
"""

import sys

sys.path.insert(0, "/opt/trn_rl_repo")

import numpy as np
import ml_dtypes

import concourse.bass as bass
import concourse.mybir as mybir
import concourse.tile as tile
from concourse.masks import make_identity
from concourse.bass_utils import run_bass_kernel_spmd

AFT = mybir.ActivationFunctionType
FP32 = mybir.dt.float32
BF16 = mybir.dt.bfloat16
FP8 = mybir.dt.float8e4
DR = mybir.MatmulPerfMode.DoubleRow

N_CORES = 8
B, S, H, I = 4, 8192, 1024, 4096
NTOK = B * S              # 32768 tokens total
T = NTOK // N_CORES       # 4096 tokens per core
TB = 512                  # tokens per block (moving free dim)
G = TB // 128             # 4 token sub-tiles per block
KH = H // 128             # 8 contraction chunks for GEMM1
MI = I // 128             # 32 I-chunks (GEMM1 out / GEMM2 contraction)
NH = H // 512             # 2 H output slices for GEMM2
EPS = 1e-5
WSC = 64.0                # host-side weight scale for fp8 range


def _split_multiwait_instructions(nc):
    """This walrus build accepts only ONE sync-wait command per instruction.
    Move extra waits onto fresh same-engine NOPs placed just before the
    offending instruction."""
    n_split = 0
    for f in nc.m.functions:
        for bb in f.blocks:
            insts = list(bb.instructions)
            new = []
            changed = False
            for inst in insts:
                si = inst.sync_info
                if si is not None and si.on_wait and len(si.on_wait) > 1:
                    waits = list(si.on_wait)
                    for w in waits[:-1]:
                        nop = mybir.InstNoOp(name=nc.get_next_instruction_name())
                        nop.engine = inst.engine
                        nop.sync_info = mybir.SyncInfo(on_wait=[w], on_update=[])
                        new.append(nop)
                        n_split += 1
                    si.on_wait = waits[-1:]
                    changed = True
                new.append(inst)
            if changed:
                bb.instructions = new
    return n_split


def _bcast_ap(ap, p=128):
    """AP view of a DRAM vector broadcast across p partitions."""
    return bass.AP(tensor=ap.tensor, offset=ap.offset, ap=[[0, p]] + list(ap.ap))


def _build(n_blocks=T // TB):
    nc = bass.Bass("TRN2")
    t_rows = n_blocks * TB
    xin = nc.declare_dram_parameter("xin", [t_rows, H], FP32, isOutput=False)
    res = nc.declare_dram_parameter("res", [t_rows, H], FP32, isOutput=False)
    # weights pre-split into column halves so the two HWDGE queues can load
    # them in parallel as single contiguous-per-partition DMAs
    w1a = nc.declare_dram_parameter("w1a", [128, KH, I // 2], FP8, isOutput=False)
    w1b = nc.declare_dram_parameter("w1b", [128, KH, I // 2], FP8, isOutput=False)
    w2a = nc.declare_dram_parameter("w2a", [128, MI, 512], FP8, isOutput=False)
    w2b = nc.declare_dram_parameter("w2b", [128, MI, 512], FP8, isOutput=False)
    b1c = nc.declare_dram_parameter("b1c", [128, MI], FP32, isOutput=False)
    b2v = nc.declare_dram_parameter("b2v", [H], FP32, isOutput=False)
    out = nc.declare_dram_parameter("out", [t_rows, H], FP32, isOutput=True)

    with tile.TileContext(nc) as tc:
        with (
            tc.tile_pool(name="const", bufs=1) as const,
            tc.tile_pool(name="ldpa", bufs=2) as ldpa,
            tc.tile_pool(name="ldpb", bufs=2) as ldpb,
            tc.tile_pool(name="blk1", bufs=1) as blk1,
            tc.tile_pool(name="outp", bufs=2) as outp,
            tc.tile_pool(name="statp", bufs=4) as statp,
            tc.tile_pool(name="ps1", bufs=2, space="PSUM") as ps1,
            tc.tile_pool(name="ps2", bufs=4, space="PSUM") as ps2,
            tc.tile_pool(name="pst", bufs=2, space="PSUM") as pst,
        ):
            # Weight halves stream on both HWDGE queues, sequenced by need:
            # sync: xa0, w1b, xb1, w2b / scalar: xb0, w1a, xa1, w2a
            w1a_sb = const.tile([128, KH, I // 2], FP8)
            w1b_sb = const.tile([128, KH, I // 2], FP8)
            w2a_sb = const.tile([128, MI, 512], FP8)
            w2b_sb = const.tile([128, MI, 512], FP8)

            b2_bc = const.tile([128, H], FP32)
            nc.gpsimd.dma_start(out=b2_bc, in_=_bcast_ap(b2v[:]))
            b1_sb = const.tile([128, MI], FP32)
            nc.gpsimd.dma_start(out=b1_sb, in_=b1c[:, :])
            eps_t = const.tile([128, 1], FP32)
            nc.vector.memset(eps_t, EPS)
            ident = const.tile([128, 128], BF16)
            make_identity(nc, ident)

            def emit_ln_load(tb, qx=None, qr=None):
                """Allocate block tiles and trigger the input loads.
                (p g) token mapping: partition p owns 4 consecutive DRAM rows
                -> 16 KB contiguous per-partition DMA descriptors (~2.7x the
                per-queue bandwidth of the 4 KB row-per-group layout)."""
                t0 = tb * TB
                tiles = {
                    "xa": ldpa.tile([128, G, H], FP32, name=f"xa_{tb}", tag="xa"),
                    "xb": ldpb.tile([128, G, H], FP32, name=f"xb_{tb}", tag="xb"),
                    "x0": blk1.tile([128, G, H], BF16, name=f"x0_{tb}", tag="x0"),
                    "xT": blk1.tile([128, KH, TB], FP8, name=f"xT_{tb}", tag="xT"),
                    "mv4": statp.tile([128, G, 2], FP32, name=f"mv_{tb}", tag="mv4"),
                    "rstd4": statp.tile([128, G], FP32, name=f"rs_{tb}", tag="rstd4"),
                }
                src_x = xin[t0 : t0 + TB, :].rearrange("(p g) c -> p g c", p=128)
                src_r = res[t0 : t0 + TB, :].rearrange("(p g) c -> p g c", p=128)
                (qx or nc.sync).dma_start(out=tiles["xa"], in_=src_x)
                (qr or nc.sync).dma_start(out=tiles["xb"], in_=src_r)
                return tiles

            def emit_ln_a(tb, pre=None, batched_sqrt=True):
                """Pre-LN sum + LN stats; ONE batched sqrt (block 0 uses
                per-group sqrt — all 4 precede the first GELU, so only one
                table swap — to shorten the startup critical path)."""
                tiles = pre if pre is not None else emit_ln_load(tb)
                nc.vector.tensor_add(out=tiles["xa"], in0=tiles["xa"], in1=tiles["xb"])
                for g in range(G):
                    stats = statp.tile([128, 2, 6], FP32, name=f"st_{tb}_{g}", tag="stats")
                    xg = tiles["xa"][:, g, :].rearrange("p (s d) -> p s d", s=2)
                    for s_ in range(2):
                        nc.vector.bn_stats(out=stats[:, s_, :], in_=xg[:, s_, :])
                    nc.vector.bn_aggr(out=tiles["mv4"][:, g, :], in_=stats)
                    if not batched_sqrt:
                        nc.scalar.activation(
                            out=tiles["rstd4"][:, g : g + 1],
                            in_=tiles["mv4"][:, g, 1:2],
                            func=AFT.Sqrt,
                            bias=eps_t,
                            scale=1.0,
                        )
                        nc.vector.reciprocal(
                            out=tiles["rstd4"][:, g : g + 1],
                            in_=tiles["rstd4"][:, g : g + 1],
                        )
                if batched_sqrt:
                    nc.scalar.activation(
                        out=tiles["rstd4"],
                        in_=tiles["mv4"][:, :, 1:2],
                        func=AFT.Sqrt,
                        bias=eps_t,
                        scale=1.0,
                    )
                    nc.vector.reciprocal(out=tiles["rstd4"], in_=tiles["rstd4"])
                return tiles

            def emit_ln_x0(tiles, g):
                nc.vector.tensor_scalar(
                    out=tiles["x0"][:, g, :],
                    in0=tiles["xa"][:, g, :],
                    scalar1=tiles["mv4"][:, g, 0:1],
                    scalar2=tiles["rstd4"][:, g : g + 1],
                    op0=mybir.AluOpType.subtract,
                    op1=mybir.AluOpType.mult,
                )

            def emit_transposes(tiles, tb):
                """PE-transpose group g's 8 H-chunks into ONE grouped PSUM
                bank, then evacuate with a single DVE copy (bf16 -> fp8)."""
                for g in range(G):
                    pt = pst.tile([128, KH, 128], BF16, name=f"pt_{tb}_{g}", tag="pt")
                    for k in range(KH):
                        nc.tensor.transpose(
                            pt[:, k, :],
                            tiles["x0"][:, g, k * 128 : (k + 1) * 128],
                            ident,
                        )
                    nc.vector.tensor_copy(
                        out=tiles["xT"][:, :, g * 128 : (g + 1) * 128], in_=pt
                    )

            def emit_gemm1(tb, tiles):
                hT = blk1.tile([128, MI, TB], FP8, name=f"hT_{tb}", tag="hT")
                for m in range(MI):
                    p1 = ps1.tile([128, TB], FP32, name=f"p1_{tb}_{m}", tag="p1")
                    w1_half = w1a_sb if m < MI // 2 else w1b_sb
                    mm = m % (MI // 2)
                    for k in range(KH // 2):
                        nc.tensor.matmul(
                            p1,
                            lhsT=w1_half[:, 2 * k : 2 * k + 2, mm * 128 : (mm + 1) * 128],
                            rhs=tiles["xT"][:, 2 * k : 2 * k + 2, :],
                            start=(k == 0),
                            stop=(k == KH // 2 - 1),
                            perf_mode=DR,
                        )
                    nc.scalar.activation(
                        out=hT[:, m, :],
                        in_=p1,
                        func=AFT.Gelu_apprx_tanh,
                        bias=b1_sb[:, m : m + 1],
                        scale=1.0 / WSC,
                    )
                tiles["hT"] = hT

            def emit_g2n(tb, n, tiles):
                hT = tiles["hT"]
                p2s = [
                    ps2.tile([128, 512], FP32, name=f"p2_{tb}_{n}_{g}", tag="p2")
                    for g in range(G)
                ]
                w2_half = w2a_sb if n == 0 else w2b_sb
                for g in range(G):
                    for k in range(MI // 2):
                        nc.tensor.matmul(
                            p2s[g],
                            lhsT=hT[:, 2 * k : 2 * k + 2, g * 128 : (g + 1) * 128],
                            rhs=w2_half[:, 2 * k : 2 * k + 2, :],
                            start=(k == 0),
                            stop=(k == MI // 2 - 1),
                            perf_mode=DR,
                        )
                return p2s

            def emit_evict_g(tb, n, g, p2s, o_b, tiles):
                sl = slice(n * 512, (n + 1) * 512)
                nc.vector.tensor_scalar(
                    out=o_b[:, g, sl],
                    in0=p2s[g],
                    scalar1=1.0 / WSC,
                    scalar2=None,
                    op0=mybir.AluOpType.mult,
                )
                nc.vector.tensor_add(
                    out=o_b[:, g, sl],
                    in0=o_b[:, g, sl],
                    in1=tiles["xa"][:, g, sl],
                )
                # output_b add on gpsimd (keeps the DVE stream short)
                nc.gpsimd.tensor_add(
                    out=o_b[:, g, sl], in0=o_b[:, g, sl], in1=b2_bc[:, sl]
                )

            def emit_store(tb, o_b):
                # one store per block on the scalar HWDGE queue; the (p g)
                # token mapping makes each partition's 4 rows contiguous in
                # DRAM -> 16 KB descriptors
                t0 = tb * TB
                dst = out[t0 : t0 + TB, :].rearrange("(p g) c -> p g c", p=128)
                nc.scalar.dma_start(out=dst, in_=o_b)

            # ---- prologue: block 0 LN; W2 trigger deferred past the sqrt ----
            # trigger order by deadline — sync: xa0, w1b, xb1, w2b;
            # scalar: xb0, w1a, xa1, w2a
            pre0 = emit_ln_load(0, qx=nc.sync, qr=nc.scalar)
            nc.scalar.dma_start(out=w1a_sb, in_=w1a[:, :, :])
            nc.sync.dma_start(out=w1b_sb, in_=w1b[:, :, :])
            pre1 = emit_ln_load(1, qx=nc.scalar, qr=nc.sync)
            nc.scalar.dma_start(out=w2a_sb, in_=w2a[:, :, :])
            nc.sync.dma_start(out=w2b_sb, in_=w2b[:, :, :])
            tiles = emit_ln_a(0, pre=pre0, batched_sqrt=False)
            for g in range(G):
                emit_ln_x0(tiles, g)
            emit_transposes(tiles, 0)

            # ---- steady state ----
            # Each block's store trigger is emitted after the NEXT block's
            # GEMM1 gelus so the in-order scalar-engine stream never blocks a
            # gelu on the store's wait condition.
            pending_store = None
            for tb in range(n_blocks):
                emit_gemm1(tb, tiles)
                if pending_store is not None:
                    emit_store(tb - 1, pending_store)
                p2s0 = emit_g2n(tb, 0, tiles)
                o_b = outp.tile([128, G, H], FP32, name=f"o_{tb}", tag="o")
                last = tb + 1 >= n_blocks
                if not last:
                    nxt = emit_ln_a(tb + 1, pre=pre1 if tb == 0 else None)
                    for g in range(G):
                        emit_ln_x0(nxt, g)
                        emit_evict_g(tb, 0, g, p2s0, o_b, tiles)
                    emit_transposes(nxt, tb + 1)
                else:
                    nxt = None
                    for g in range(G):
                        emit_evict_g(tb, 0, g, p2s0, o_b, tiles)
                    # last block: stream the n=0 half out eagerly (sync queue
                    # is idle by now) so only the n=1 half trails the PE
                    t0 = tb * TB
                    nc.sync.dma_start(
                        out=out[t0 : t0 + TB, 0:512].rearrange(
                            "(p g) c -> p g c", p=128
                        ),
                        in_=o_b[:, :, 0:512],
                    )
                p2s1 = emit_g2n(tb, 1, tiles)
                for g in range(G):
                    emit_evict_g(tb, 1, g, p2s1, o_b, tiles)
                if last:
                    t0 = tb * TB
                    nc.scalar.dma_start(
                        out=out[t0 : t0 + TB, 512:H].rearrange(
                            "(p g) c -> p g c", p=128
                        ),
                        in_=o_b[:, :, 512:H],
                    )
                    pending_store = None
                else:
                    pending_store = o_b
                tiles = nxt

    return nc


def _prep_inputs(input, residual, bias, attn_nw, attn_nb, inter_w, inter_b, output_w, output_b):
    """Host-side preprocessing: fold bias into the input stream, LN affine
    into W1/b1; scale weights by 64, cast to fp8 e4m3, pre-permute to the
    SBUF-resident layouts."""
    f8 = ml_dtypes.float8_e4m3
    x2 = np.ascontiguousarray(
        np.asarray(input, np.float32).reshape(NTOK, H)
        + np.asarray(bias, np.float32)[None, :]
    )
    r2 = np.ascontiguousarray(np.asarray(residual, np.float32).reshape(NTOK, H))
    gamma = np.asarray(attn_nw, np.float64)
    beta = np.asarray(attn_nb, np.float64)
    w1f = np.asarray(inter_w, np.float64)
    w1q = (gamma[:, None] * w1f * WSC).astype(np.float32).astype(f8)
    # [H, I] -> [128, KH, I] with h = k*128 + p; split into column halves
    w1d = w1q.reshape(KH, 128, I).transpose(1, 0, 2)
    w1da = np.ascontiguousarray(w1d[:, :, : I // 2])
    w1db = np.ascontiguousarray(w1d[:, :, I // 2 :])
    b1p = (np.asarray(inter_b, np.float64) + beta @ w1f).astype(np.float32)
    b1c = np.ascontiguousarray(b1p.reshape(MI, 128).T)
    w2q = (np.asarray(output_w, np.float64) * WSC).astype(np.float32).astype(f8)
    # [I, H] -> [128, MI, H] with i = m*128 + p; split into column halves
    w2d = w2q.reshape(MI, 128, H).transpose(1, 0, 2)
    w2da = np.ascontiguousarray(w2d[:, :, :512])
    w2db = np.ascontiguousarray(w2d[:, :, 512:])
    b2f = np.asarray(output_b, np.float32)

    in_maps = []
    for c in range(N_CORES):
        sl = slice(c * T, (c + 1) * T)
        in_maps.append(
            {
                "xin": x2[sl],
                "res": r2[sl],
                "w1a": w1da,
                "w1b": w1db,
                "w2a": w2da,
                "w2b": w2db,
                "b1c": b1c,
                "b2v": b2f,
            }
        )
    return in_maps


def _run(inputs, trace=False, **kwargs):
    in_maps = _prep_inputs(
        inputs["input"],
        inputs["residual"],
        inputs["bias"],
        inputs["attn_nw"],
        inputs["attn_nb"],
        inputs["inter_w"],
        inputs["inter_b"],
        inputs["output_w"],
        inputs["output_b"],
    )
    nc = _build()
    _split_multiwait_instructions(nc)
    r = run_bass_kernel_spmd(nc, in_maps, list(range(N_CORES)), trace=trace, **kwargs)
    outs = [r.results[c]["out"] for c in range(N_CORES)]
    full = np.concatenate(outs, axis=0).reshape(B, S, H).astype(np.float32)
    return full, r


def kernel(**inputs):
    out, _ = _run(inputs, trace=False)
    return out


if __name__ == "__main__":
    nc = _build(1)
    print("built 1-block variant ok:", len(nc.m.functions[0].blocks))
